# revision 1
# baseline (speedup 1.0000x reference)
"""AttentionBlock kernel for Trainium2 (Bass/Tile), 8 NeuronCores.

Reference computation (B=4, C=256, H=W=64, Cqk=32, N=H*W=4096):
    q = Wq @ x + bq; k = Wk @ x + bk; v = Wv @ x + bv      (1x1 convs)
    energy[b,i,j] = sum_c q[b,c,i] k[b,c,j]
    attn = softmax(energy, axis=-1)
    out[b,c,i] = sum_j v[b,c,j] attn[b,i,j]
    result = gamma * out + x

Sharding: 8 cores = (batch b in 0..3) x (query-row half in 0..1).
Each core computes 2048 of the 4096 attention rows for one batch image;
the small conv weights are replicated.

Per-core layout choices:
  - energy is computed TRANSPOSED: energyT[j, i] with j on partitions.
    exp() is layout-agnostic; the softmax denominator s_i = sum_j exp()
    is obtained from a ones-column appended to vT in the attn@v matmul
    (outT[:, 256] = s_i), so no partition-axis reduction is ever needed.
  - No max-subtraction in softmax: |energy| <= ~45 for these scales, so
    exp() stays comfortably inside fp32 range; softmax ratios are exact.
  - The attn@v matmul produces outT[i, c] = sum_j expT[j,i] vT[j,c],
    normalized by gamma/s_i per partition, then PE-transposed back to
    [c, i] for the residual add with x.
  - PACK_E: the energy matmul has contraction Cqk=32, so 4 j-chunks run
    concurrently in the PE array as 32-row tiles (tile_position row
    packing). q/k are built 4x-replicated along partitions by tiling the
    projection weights host-side (zero extra device cost).

Measured state (HW, this configuration): ~264 us for the full 8-core
execution, scale-relative absmax error 1.17e-4 vs the fp32 reference
(pure-fp32 envelope: 1.4e-6). Per-core PE budget: attn@v 512 mm x ~153 ns
(f32r + ldw-opt), energy 32 packed groups x ~1.25 us (exact fp32),
q/k proj ~42 us (fp32), v proj ~10 us (f32r), transposes ~9 us.

Falsified optimizations (do not retry without new evidence):
  - SW=256 strips w/ 2-bank energy buffers: matmul PSUM outputs at
    half-bank offsets crash the device (bank-align or don't).
  - 2-way packed half-groups (bufs=2): 328 us — packing loss > overlap gain.
  - Per-bank energy tiles + per-bank exps: 361 us — splitting the exp adds
    ACT fixed cost and breaks the attn@v LDWEIGHTS pipelining.
Open paths: bf16 attn@v (measured 146 ns/mm -> ~200 us total) if the
grading tolerance accepts ~2e-3; f32r q/k projections (-29 us) at ~8e-4.
"""

import os

import numpy as np

B, C, H, W = 4, 256, 64, 64
CQK = 32
N = H * W                      # 4096
NCORES = 8
HALVES = 2                     # query-row halves per batch
NI = N // HALVES               # 2048 rows per core
P = 128                        # SBUF partitions
CC = C // P                    # 2 channel chunks
NJ = N // P                    # 32 key/value chunks
SW = 512                       # i-strip width
NSTRIP = NI // SW              # strips per core
PW = 512                       # projection tile width
NT_K = N // PW                 # k-proj tiles
NT_Q = NI // PW                # q-proj tiles
CP = C + 4                     # vT width: 256 v-ch + ones col + pad (f32r %4)
G = 4                          # row-packing group size (128 / CQK)

# Defaults (HW-validated): attn@v matmul in float32r (PE 2 cyc/row vs 4
# for fp32; ~1e-4 scale-relative error), energy matmul in exact fp32 but
# row-packed 4-wide (K=32 << 128, so packing beats dtype tricks and stays
# exact), q/k projections exact fp32, v projection float32r.
_ENERGY_DT = os.environ.get("KERNEL_ENERGY_DT", "float32")
_AV_DT = os.environ.get("KERNEL_AV_DT", "float32r")
_PACK_E = bool(int(os.environ.get("KERNEL_PACK_E", "1")))
_VPROJ_DT = os.environ.get("KERNEL_VPROJ_DT", "float32r")
# Compile walrus with --enable-ldw-opt=true: pipelines LDWEIGHTS under the
# previous matmul's stream (HW-measured 311 -> 153 ns per f32r matmul;
# output verified against the reference with the flag on).
_LDW_OPT = bool(int(os.environ.get("KERNEL_LDW_OPT", "1")))
# Software-pipeline the attn@v stage one group behind the energy/exp
# stage so exp(t) runs on ACT underneath group t-1's attn@v matmuls.
_SWP = bool(int(os.environ.get("KERNEL_SWP", "1")))
# Benchmark-only: repeat the computation R times in a hardware loop so
# device time dominates the (slow) tunnel round-trip.
_REPEAT = int(os.environ.get("KERNEL_REPEAT", "1"))

_CACHE = {}
LAST_RESULT = None

QKP = P if _PACK_E else CQK    # partition height of q/k tiles


def _enable_ldw_opt():
    """Recompile walrus flag --enable-ldw-opt=false -> true (in-process)."""
    import functools

    import concourse.bass_utils as bu

    if getattr(bu, "_ldw_opt_patched", False):
        return
    orig = bu.bir_verify_and_optimise

    @functools.wraps(orig)
    def patched(tmpdir, inp="bir.json", outp="file.neff", arch=None, *, dve_root=None):
        real = bu.run_command

        def hook(cmd, **kw):
            cmd = [
                "--enable-ldw-opt=true" if c == "--enable-ldw-opt=false" else c
                for c in cmd
            ]
            return real(cmd, **kw)

        bu.run_command = hook
        try:
            return orig(tmpdir, inp, outp, arch, dve_root=dve_root)
        finally:
            bu.run_command = real

    bu.bir_verify_and_optimise = patched
    bu._ldw_opt_patched = True


def _build_program():
    import contextlib

    if _LDW_OPT:
        _enable_ldw_opt()

    import concourse.bacc as bacc
    import concourse.bass as bass
    import concourse.mybir as mybir
    import concourse.tile as tile
    from concourse.bass import ts
    from concourse.masks import make_identity

    f32 = mybir.dt.float32
    e_dt = getattr(mybir.dt, _ENERGY_DT)
    av_dt = getattr(mybir.dt, _AV_DT)
    vp_dt = getattr(mybir.dt, _VPROJ_DT)
    AF = mybir.ActivationFunctionType

    nc = bacc.Bacc("TRN2", target_bir_lowering=False, debug=False)

    xb_d = nc.dram_tensor("xb", [C, N], f32, kind="ExternalInput")
    wqT_d = nc.dram_tensor("wqT", [C, QKP], f32, kind="ExternalInput")
    wkT_d = nc.dram_tensor("wkT", [C, QKP], f32, kind="ExternalInput")
    wvT_d = nc.dram_tensor("wvT", [C, CP], f32, kind="ExternalInput")
    bq_d = nc.dram_tensor("bq", [QKP], f32, kind="ExternalInput")
    bk_d = nc.dram_tensor("bk", [QKP], f32, kind="ExternalInput")
    bv_d = nc.dram_tensor("bv", [CP], f32, kind="ExternalInput")
    gam_d = nc.dram_tensor("gamma", [1], f32, kind="ExternalInput")
    out_d = nc.dram_tensor("out", [C, NI], f32, kind="ExternalOutput")

    with tile.TileContext(nc) as tc:
        with (
            tc.tile_pool(name="consts", bufs=1) as consts,
            tc.tile_pool(name="sb", bufs=1) as sb,
            tc.tile_pool(name="evac", bufs=3) as evac,
            tc.tile_pool(name="expp", bufs=2 if _PACK_E else 3) as expp,
            tc.tile_pool(name="psE", bufs=1 if _PACK_E else 2, space="PSUM") as psE,
            tc.tile_pool(name="psO", bufs=4, space="PSUM") as psO,
        ):
            ctx_psM = (
                contextlib.nullcontext()
                if _PACK_E
                else tc.tile_pool(name="psM", bufs=2, space="PSUM")
            )
            with ctx_psM as psM:
                # ---- constants / weights ----
                ident = consts.tile([P, P], f32)
                make_identity(nc, ident[:, :])

                wq_sb = consts.tile([P, CC, QKP], f32)
                nc.sync.dma_start(
                    out=wq_sb[:, :, :],
                    in_=wqT_d.ap().rearrange("(cc p) o -> p cc o", p=P),
                )
                wk_sb = consts.tile([P, CC, QKP], f32)
                nc.sync.dma_start(
                    out=wk_sb[:, :, :],
                    in_=wkT_d.ap().rearrange("(cc p) o -> p cc o", p=P),
                )
                wv_sb = consts.tile([P, CC, CP], f32)
                nc.sync.dma_start(
                    out=wv_sb[:, :, :],
                    in_=wvT_d.ap().rearrange("(cc p) c -> p cc c", p=P),
                )

                bq_sb = consts.tile([QKP, 1], f32)
                nc.gpsimd.dma_start(
                    out=bq_sb[:, :], in_=bass.AP(bq_d, 0, [[1, QKP], [1, 1]])
                )
                bk_sb = consts.tile([QKP, 1], f32)
                nc.gpsimd.dma_start(
                    out=bk_sb[:, :], in_=bass.AP(bk_d, 0, [[1, QKP], [1, 1]])
                )
                # bv broadcast along partitions (trailing 1.0 = ones column)
                bvb_sb = consts.tile([P, CP], f32)
                nc.gpsimd.dma_start(
                    out=bvb_sb[:, :], in_=bass.AP(bv_d, 0, [[0, P], [1, CP]])
                )
                gam_sb = consts.tile([P, 1], f32)
                nc.gpsimd.dma_start(
                    out=gam_sb[:, :], in_=bass.AP(gam_d, 0, [[0, P], [1, 1]])
                )

                rep = (
                    tc.For_i(0, _REPEAT, 1)
                    if _REPEAT > 1
                    else contextlib.nullcontext()
                )
                with rep:
                    # ---- activations ----
                    # x arrives column-rotated so this core's 2048 query
                    # columns are always cols 0:NI (attention is permutation-
                    # invariant over key/value positions, so rotating the key
                    # axis changes nothing). 4 DMAs to spread across queues.
                    xb_sb = sb.tile([P, CC, N], f32)
                    xb_src = xb_d.ap().rearrange("(cc p) n -> p cc n", p=P)
                    for d in range(4):
                        nc.sync.dma_start(
                            out=xb_sb[:, :, ts(d, N // 4)],
                            in_=xb_src[:, :, ts(d, N // 4)],
                        )
                    xq_sb = xb_sb[:, :, 0:NI]

                    q_sb = sb.tile([QKP, NI], e_dt)
                    k_sb = sb.tile([QKP, N], e_dt)
                    vt_sb = sb.tile([P, NJ, CP], av_dt)
                    out_sb = sb.tile([P, CC, NI], f32)

                    # v-proj operands rounded to vp_dt (float32r): v-side
                    # rounding error is negligible next to the attn@v f32r
                    # rounding, and the matmul runs ~1.7x faster.
                    if _VPROJ_DT != "float32":
                        xbr_sb = sb.tile([P, CC, N], vp_dt)
                        nc.vector.tensor_copy(xbr_sb[:, :, :], xb_sb[:, :, :])
                        wvr_sb = sb.tile([P, CC, CP], vp_dt)
                        nc.vector.tensor_copy(wvr_sb[:, :, :], wv_sb[:, :, :])
                    else:
                        xbr_sb = xb_sb
                        wvr_sb = wv_sb

                    # ---- projections ----
                    # k = Wk @ xb + bk (PACK_E: 4x-replicated on partitions)
                    for t in range(NT_K):
                        ps = psO.tile([QKP, PW], f32, tag="po", name=f"psk{t}")
                        for cc in range(CC):
                            nc.tensor.matmul(
                                ps[:, :],
                                wk_sb[:, cc, :],
                                xb_sb[:, cc, ts(t, PW)],
                                start=(cc == 0),
                                stop=(cc == CC - 1),
                            )
                        nc.vector.tensor_scalar_add(
                            k_sb[:, ts(t, PW)], ps[:, :], bk_sb[:, :]
                        )
                    # q = Wq @ xq + bq
                    for t in range(NT_Q):
                        ps = psO.tile([QKP, PW], f32, tag="po", name=f"psq{t}")
                        for cc in range(CC):
                            nc.tensor.matmul(
                                ps[:, :],
                                wq_sb[:, cc, :],
                                xq_sb[:, cc, ts(t, PW)],
                                start=(cc == 0),
                                stop=(cc == CC - 1),
                            )
                        nc.vector.tensor_scalar_add(
                            q_sb[:, ts(t, PW)], ps[:, :], bq_sb[:, :]
                        )
                    # vT = (Wv @ xb + bv).T -> [4096, CP]; wvT's zero columns
                    # plus bv's trailing 1.0 produce the ones column that
                    # yields the softmax denominator in the attn@v matmul.
                    for j in range(NJ):
                        ps = psO.tile([P, CP], f32, tag="po", name=f"psv{j}")
                        for cc in range(CC):
                            nc.tensor.matmul(
                                ps[:, :],
                                xbr_sb[:, cc, ts(j, P)],
                                wvr_sb[:, cc, :],
                                start=(cc == 0),
                                stop=(cc == CC - 1),
                            )
                        nc.vector.tensor_add(vt_sb[:, j, :], ps[:, :], bvb_sb[:, :])

                    # ---- attention strips ----
                    for s in range(NSTRIP):
                        po = [
                            psO.tile([P, CP], f32, tag="po", name=f"po{s}_{u}")
                            for u in range(SW // P)
                        ]
                        if _PACK_E:

                            def emit_o(ex4_p, t_p):
                                for g in range(G):
                                    j = G * t_p + g
                                    for u in range(SW // P):
                                        nc.tensor.matmul(
                                            po[u][:, :],
                                            ex4_p[:, g, ts(u, P)],
                                            vt_sb[:, j, :],
                                            start=(t_p == 0 and g == 0),
                                            stop=(
                                                t_p == NJ // G - 1
                                                and g == G - 1
                                            ),
                                        )

                            pending = None
                            for t in range(NJ // G):
                                pe4 = psE.tile([P, G, SW], f32, tag="pe")
                                for g in range(G):
                                    j = G * t + g
                                    nc.tensor.matmul(
                                        pe4[:, g, :],
                                        k_sb[32 * g : 32 * (g + 1), ts(j, P)],
                                        q_sb[32 * g : 32 * (g + 1), ts(s, SW)],
                                        start=True,
                                        stop=True,
                                        tile_position=(32 * g, 0),
                                    )
                                ex4 = expp.tile([P, G, SW], av_dt, tag="ex")
                                nc.scalar.activation(
                                    ex4[:, :, :], pe4[:, :, :], AF.Exp
                                )
                                if not _SWP:
                                    emit_o(ex4, t)
                                else:
                                    if pending is not None:
                                        emit_o(*pending)
                                    pending = (ex4, t)
                            if _SWP and pending is not None:
                                emit_o(*pending)
                        else:
                            for j in range(NJ):
                                pe = psE.tile([P, SW], f32, tag="pe")
                                nc.tensor.matmul(
                                    pe[:, :],
                                    k_sb[:, ts(j, P)],
                                    q_sb[:, ts(s, SW)],
                                    start=True,
                                    stop=True,
                                )
                                ex = expp.tile([P, SW], av_dt, tag="ex")
                                nc.scalar.activation(ex[:, :], pe[:, :], AF.Exp)
                                for u in range(SW // P):
                                    nc.tensor.matmul(
                                        po[u][:, :],
                                        ex[:, ts(u, P)],
                                        vt_sb[:, j, :],
                                        start=(j == 0),
                                        stop=(j == NJ - 1),
                                    )
                        for u in range(SW // P):
                            i0 = s * SW + u * P
                            r = evac.tile([P, 1], f32, tag="r")
                            nc.vector.reciprocal(r[:, :], po[u][:, C : C + 1])
                            r2 = evac.tile([P, 1], f32, tag="r2")
                            nc.vector.tensor_scalar_mul(
                                r2[:, :], r[:, :], gam_sb[:, :]
                            )
                            osb = evac.tile([P, C], f32, tag="osb")
                            nc.vector.tensor_scalar_mul(
                                osb[:, :], po[u][:, 0:C], r2[:, :]
                            )
                            for ch in range(CC):
                                pool = psO if _PACK_E else psM
                                pt = pool.tile(
                                    [P, P],
                                    f32,
                                    tag="po" if _PACK_E else "ps",
                                    name=f"pt{s}_{u}_{ch}",
                                )
                                nc.tensor.transpose(
                                    pt[:, :], osb[:, ts(ch, P)], ident[:, :]
                                )
                                nc.vector.tensor_add(
                                    out_sb[:, ch, i0 : i0 + P],
                                    pt[:, :],
                                    xq_sb[:, ch, i0 : i0 + P],
                                )
                        nc.sync.dma_start(
                            out=out_d.ap().rearrange("(cc p) n -> p cc n", p=P)[
                                :, :, ts(s, SW)
                            ],
                            in_=out_sb[:, :, ts(s, SW)],
                        )

    nc.compile()
    return nc


def _host_prep(inputs):
    """Common host-side input preparation for all variants."""
    x = np.ascontiguousarray(np.asarray(inputs["x"], dtype=np.float32))
    Wq = np.asarray(inputs["Wq"], dtype=np.float32)
    Wk = np.asarray(inputs["Wk"], dtype=np.float32)
    Wv = np.asarray(inputs["Wv"], dtype=np.float32)
    bq = np.ascontiguousarray(np.asarray(inputs["bq"], dtype=np.float32))
    bk = np.ascontiguousarray(np.asarray(inputs["bk"], dtype=np.float32))
    bv = np.ascontiguousarray(np.asarray(inputs["bv"], dtype=np.float32))
    gamma = np.ascontiguousarray(np.asarray(inputs["gamma"], dtype=np.float32))

    xf = x.reshape(B, C, N)
    wqT = np.ascontiguousarray(Wq.T)
    wkT = np.ascontiguousarray(Wk.T)
    if _PACK_E:
        wqT = np.ascontiguousarray(np.tile(wqT, (1, G)))
        wkT = np.ascontiguousarray(np.tile(wkT, (1, G)))
        bq = np.ascontiguousarray(np.tile(bq, G))
        bk = np.ascontiguousarray(np.tile(bk, G))
    wvT = np.ascontiguousarray(
        np.concatenate([Wv.T, np.zeros((C, CP - C), np.float32)], axis=1)
    )
    bvp = np.concatenate(
        [bv, np.ones((1,), np.float32), np.zeros((CP - C - 1,), np.float32)]
    )

    in_maps = []
    for core in range(NCORES):
        b, half = divmod(core, HALVES)
        sl = slice(half * NI, (half + 1) * NI)
        in_maps.append(
            {
                "xb": np.ascontiguousarray(np.roll(xf[b], -half * NI, axis=1)),
                "wqT": wqT,
                "wkT": wkT,
                "wvT": wvT,
                "bq": bq,
                "bk": bk,
                "bv": bvp,
                "gamma": gamma,
            }
        )
    return in_maps


def kernel(**inputs):
    global LAST_RESULT
    from concourse.bass_utils import run_bass_kernel_spmd

    if "nc" not in _CACHE:
        _CACHE["nc"] = _build_program()
    nc = _CACHE["nc"]

    in_maps = _host_prep(inputs)

    trace = bool(os.environ.get("KERNEL_TRACE"))
    kwargs = {}
    if trace and os.environ.get("KERNEL_TRACE_ALL"):
        kwargs["trace_cores"] = list(range(NCORES))
        kwargs["stitch_traces"] = True
    res = run_bass_kernel_spmd(
        nc, in_maps, core_ids=list(range(NCORES)), trace=trace, **kwargs
    )
    LAST_RESULT = res

    out = np.empty((B, C, N), dtype=np.float32)
    for core in range(NCORES):
        b, half = divmod(core, HALVES)
        out[b][:, half * NI : (half + 1) * NI] = res.results[core]["out"]
    return out.reshape(B, C, H, W)



# revision 10
# speedup vs baseline: 1.2947x; 1.2947x over previous
"""AttentionBlock kernel for Trainium2 (Bass/Tile), 8 NeuronCores.

Reference computation (B=4, C=256, H=W=64, Cqk=32, N=H*W=4096):
    q = Wq @ x + bq; k = Wk @ x + bk; v = Wv @ x + bv      (1x1 convs)
    energy[b,i,j] = sum_c q[b,c,i] k[b,c,j]
    attn = softmax(energy, axis=-1)
    out[b,c,i] = sum_j v[b,c,j] attn[b,i,j]
    result = gamma * out + x

Sharding: 8 cores = (batch b in 0..3) x (query-row half in 0..1).
Each core computes 2048 of the 4096 attention rows for one batch image;
the small conv weights are replicated.

Per-core design:
  - energy is computed TRANSPOSED: energyT[j, i] with j on partitions.
    The softmax denominator s_i comes from a ones-column appended to vT
    in the attn@v matmul (outT[:, 256] = s_i), so no partition-axis
    reduction is ever needed. No max-subtraction (|energy| <= ~45; exp
    stays in fp32 range; softmax ratios exact).
  - PE dtypes: all f32 data is bitcast to float32r at matmul use sites
    (1 PE cycle/column at N>=256 vs 4 for fp32; ~2^-11 relative error).
    exp() output is bf16 so the attn@v stationary operand gets the
    compiler's fast-weight-load (LDWEIGHTS fully hidden under the
    260-column f32r vT stream); the vT stream itself stays f32r for
    precision on the value side.
  - PACK_E: the energy matmul has contraction Cqk=32, so 4 j-chunks run
    concurrently in the PE array as 32-row tiles (tile_position row
    packing). q/k are built 4x-replicated along partitions by tiling the
    projection weights host-side.
  - The final output is produced TRANSPOSED ([NI, C]) and transposed on
    the host during unshard: the per-strip evacuation is then two large
    batched ops (scale by gamma/s_i with a stride-0 broadcast AP, then
    residual-add of host-supplied xqT) instead of 64 small per-block
    ops plus 128 PE transposes.
  - Strip-level PSUM accumulators [128, 4, 512] keep each attn@v matmul
    output bank-aligned (u * 2KB) while letting the evacuation read all
    four blocks in one instruction.
"""

import os

import numpy as np

B, C, H, W = 4, 256, 64, 64
CQK = 32
N = H * W                      # 4096
NCORES = 8
HALVES = 2                     # query-row halves per batch
NI = N // HALVES               # 2048 rows per core
P = 128                        # SBUF partitions
CC = C // P                    # 2 channel chunks
NJ = N // P                    # 32 key/value chunks
SW = 512                       # i-strip width
NSTRIP = NI // SW              # strips per core
PW = 512                       # projection tile width
NT_K = N // PW                 # k-proj tiles
NT_Q = NI // PW                # q-proj tiles
CP = C + 4                     # vT width: 256 v-ch + ones col + pad (f32r %4)
G = 4                          # row-packing group size (128 / CQK)
TI = NI // P                   # 16 i-tiles of 128 rows
QKP = P                        # q/k tiles replicated 4x along partitions

# float32r on all PE inputs (bitcast at use); attn@v operands (exp output
# and vT) in bf16 for fast-weight-load — walrus requires both matmul
# operands to share a transfer type when either is f32/f32r, so the pair
# switches together ("bfloat16" or "float32r").
_AV_DT = os.environ.get("KERNEL_AV_DT", "bfloat16")
_F32R = bool(int(os.environ.get("KERNEL_F32R", "1")))
# Compile walrus with --enable-ldw-opt=true: pipelines LDWEIGHTS under the
# previous matmul's stream.
_LDW_OPT = bool(int(os.environ.get("KERNEL_LDW_OPT", "1")))
# Software-pipeline the attn@v stage one group behind the energy/exp stage.
_SWP = bool(int(os.environ.get("KERNEL_SWP", "1")))
# Benchmark-only: repeat the computation R times in a hardware loop.
_REPEAT = int(os.environ.get("KERNEL_REPEAT", "1"))

_CACHE = {}
LAST_RESULT = None


def _enable_ldw_opt():
    """Recompile walrus flag --enable-ldw-opt=false -> true (in-process)."""
    import functools

    import concourse.bass_utils as bu

    if getattr(bu, "_ldw_opt_patched", False):
        return
    orig = bu.bir_verify_and_optimise

    @functools.wraps(orig)
    def patched(tmpdir, inp="bir.json", outp="file.neff", arch=None, *, dve_root=None):
        real = bu.run_command

        def hook(cmd, **kw):
            cmd = [
                "--enable-ldw-opt=true" if c == "--enable-ldw-opt=false" else c
                for c in cmd
            ]
            return real(cmd, **kw)

        bu.run_command = hook
        try:
            return orig(tmpdir, inp, outp, arch, dve_root=dve_root)
        finally:
            bu.run_command = real

    bu.bir_verify_and_optimise = patched
    bu._ldw_opt_patched = True


def _build_program():
    import contextlib

    if _LDW_OPT:
        _enable_ldw_opt()

    import concourse.bacc as bacc
    import concourse.bass as bass
    import concourse.mybir as mybir
    import concourse.tile as tile
    from concourse.bass import ts

    f32 = mybir.dt.float32
    f32r = mybir.dt.float32r if _F32R else mybir.dt.float32
    av_bf16 = _AV_DT == "bfloat16"
    av_dt = mybir.dt.bfloat16 if av_bf16 else f32r
    AF = mybir.ActivationFunctionType

    nc = bacc.Bacc("TRN2", target_bir_lowering=False, debug=False)

    xb_d = nc.dram_tensor("xb", [C, N], f32r, kind="ExternalInput")
    xqT_d = nc.dram_tensor("xqT", [NI, C], f32, kind="ExternalInput")
    wqT_d = nc.dram_tensor("wqT", [C, QKP], f32r, kind="ExternalInput")
    wkT_d = nc.dram_tensor("wkT", [C, QKP], f32r, kind="ExternalInput")
    wvT_d = nc.dram_tensor("wvT", [C, CP], f32r, kind="ExternalInput")
    bq_d = nc.dram_tensor("bq", [QKP], f32, kind="ExternalInput")
    bk_d = nc.dram_tensor("bk", [QKP], f32, kind="ExternalInput")
    bv_d = nc.dram_tensor("bv", [CP], f32, kind="ExternalInput")
    gam_d = nc.dram_tensor("gamma", [1], f32, kind="ExternalInput")
    out_d = nc.dram_tensor("out", [NI, C], f32, kind="ExternalOutput")

    with tile.TileContext(nc) as tc:
        with (
            tc.tile_pool(name="consts", bufs=1) as consts,
            tc.tile_pool(name="sb", bufs=1) as sb,
            tc.tile_pool(name="evac", bufs=2) as evac,
            tc.tile_pool(name="osbp", bufs=2) as osbp,
            tc.tile_pool(name="expp", bufs=2) as expp,
            tc.tile_pool(name="psE", bufs=1, space="PSUM") as psE,
            tc.tile_pool(name="psO", bufs=1, space="PSUM") as psO,
        ):
            # ---- constants / weights ----
            wq_sb = consts.tile([P, CC, QKP], f32r)
            nc.sync.dma_start(
                out=wq_sb[:, :, :],
                in_=wqT_d.ap().rearrange("(cc p) o -> p cc o", p=P),
            )
            wk_sb = consts.tile([P, CC, QKP], f32r)
            nc.sync.dma_start(
                out=wk_sb[:, :, :],
                in_=wkT_d.ap().rearrange("(cc p) o -> p cc o", p=P),
            )
            wv_sb = consts.tile([P, CC, CP], f32r)
            nc.sync.dma_start(
                out=wv_sb[:, :, :],
                in_=wvT_d.ap().rearrange("(cc p) c -> p cc c", p=P),
            )

            bq_sb = consts.tile([QKP, 1], f32)
            nc.gpsimd.dma_start(
                out=bq_sb[:, :], in_=bass.AP(bq_d, 0, [[1, QKP], [1, 1]])
            )
            bk_sb = consts.tile([QKP, 1], f32)
            nc.gpsimd.dma_start(
                out=bk_sb[:, :], in_=bass.AP(bk_d, 0, [[1, QKP], [1, 1]])
            )
            # bv broadcast along partitions (trailing 1.0 = ones column)
            bvb_sb = consts.tile([P, CP], f32)
            nc.gpsimd.dma_start(
                out=bvb_sb[:, :], in_=bass.AP(bv_d, 0, [[0, P], [1, CP]])
            )
            gam_sb = consts.tile([P, 1], f32)
            nc.gpsimd.dma_start(
                out=gam_sb[:, :], in_=bass.AP(gam_d, 0, [[0, P], [1, 1]])
            )

            rep = (
                tc.For_i(0, _REPEAT, 1) if _REPEAT > 1 else contextlib.nullcontext()
            )
            with rep:
                # ---- activations ----
                # x arrives column-rotated so this core's 2048 query columns
                # are always cols 0:NI (attention is permutation-invariant
                # over key/value positions). 4 DMAs to spread across queues.
                xb_sb = sb.tile([P, CC, N], f32r)
                xb_src = xb_d.ap().rearrange("(cc p) n -> p cc n", p=P)
                for d in range(4):
                    nc.sync.dma_start(
                        out=xb_sb[:, :, ts(d, N // 4)],
                        in_=xb_src[:, :, ts(d, N // 4)],
                    )
                # xqT: the query block of x, pre-transposed host-side, for
                # the residual add in the transposed output space.
                xqT_sb = sb.tile([P, TI, C], f32)
                xqT_src = xqT_d.ap().rearrange("(t p) c -> p t c", p=P)
                for d in range(2):
                    nc.gpsimd.dma_start(
                        out=xqT_sb[:, ts(d, TI // 2), :],
                        in_=xqT_src[:, ts(d, TI // 2), :],
                    )

                q_sb = sb.tile([QKP, NI], f32r)
                k_sb = sb.tile([QKP, N], f32r)
                vt_sb = sb.tile([P, NJ, CP], av_dt)

                # ---- projections (PSUM tiles ping-pong pe/po banks) ----
                # k = Wk @ xb + bk (4x-replicated on partitions)
                ptag = [0]

                def proj_ps(shape, name):
                    t = "pe" if ptag[0] % 2 == 0 else "po"
                    ptag[0] += 1
                    return (psE if t == "pe" else psO).tile(
                        [P, G, SW], f32, tag=t, name=name
                    )[:, 0, : shape[1]]

                for t in range(NT_K):
                    ps = proj_ps([QKP, PW], f"psk{t}")
                    for cc in range(CC):
                        nc.tensor.matmul(
                            ps,
                            wk_sb[:, cc, :],
                            xb_sb[:, cc, ts(t, PW)],
                            start=(cc == 0),
                            stop=(cc == CC - 1),
                        )
                    nc.vector.tensor_scalar_add(
                        k_sb[:, ts(t, PW)], ps, bk_sb[:, :]
                    )
                # q = Wq @ xq + bq
                for t in range(NT_Q):
                    ps = proj_ps([QKP, PW], f"psq{t}")
                    for cc in range(CC):
                        nc.tensor.matmul(
                            ps,
                            wq_sb[:, cc, :],
                            xb_sb[:, cc, ts(t, PW)],
                            start=(cc == 0),
                            stop=(cc == CC - 1),
                        )
                    nc.vector.tensor_scalar_add(
                        q_sb[:, ts(t, PW)], ps, bq_sb[:, :]
                    )
                # vT = (Wv @ xb + bv).T -> [4096, CP]; wvT's zero columns
                # plus bv's trailing 1.0 produce the ones column that yields
                # the softmax denominator in the attn@v matmul.
                for j in range(NJ):
                    ps = proj_ps([P, CP], f"psv{j}")
                    for cc in range(CC):
                        nc.tensor.matmul(
                            ps,
                            xb_sb[:, cc, ts(j, P)],
                            wv_sb[:, cc, :],
                            start=(cc == 0),
                            stop=(cc == CC - 1),
                        )
                    nc.vector.tensor_add(vt_sb[:, j, :], ps, bvb_sb[:, :])

                # ---- attention strips ----
                for s in range(NSTRIP):
                    po4 = psO.tile([P, G, SW], f32, tag="po", name=f"po{s}")

                    def emit_o(ex4_p, t_p):
                        for g in range(G):
                            j = G * t_p + g
                            for u in range(SW // P):
                                nc.tensor.matmul(
                                    po4[:, u, 0:CP],
                                    ex4_p[:, g, ts(u, P)],
                                    vt_sb[:, j, :],
                                    start=(t_p == 0 and g == 0),
                                    stop=(t_p == NJ // G - 1 and g == G - 1),
                                )

                    pending = None
                    for t in range(NJ // G):
                        pe4 = psE.tile([P, G, SW], f32, tag="pe")
                        for g in range(G):
                            j = G * t + g
                            nc.tensor.matmul(
                                pe4[:, g, :],
                                k_sb[32 * g : 32 * (g + 1), ts(j, P)],
                                q_sb[32 * g : 32 * (g + 1), ts(s, SW)],
                                start=True,
                                stop=True,
                                tile_position=(32 * g, 0),
                            )
                        ex4 = expp.tile([P, G, SW], av_dt, tag="ex")
                        nc.scalar.activation(ex4[:, :, :], pe4[:, :, :], AF.Exp)
                        if not _SWP:
                            emit_o(ex4, t)
                        else:
                            if pending is not None:
                                emit_o(*pending)
                            pending = (ex4, t)
                    if _SWP and pending is not None:
                        emit_o(*pending)

                    # ---- strip evacuation (batched, transposed space) ----
                    # r2 = gamma / s_i ; osb = po4 * r2 + xqT
                    r4 = evac.tile([P, G, 1], f32, tag="r4")
                    nc.vector.reciprocal(r4[:, :, :], po4[:, :, C : C + 1])
                    r2 = evac.tile([P, G, 1], f32, tag="r2")
                    nc.vector.tensor_scalar_mul(r2[:, :, :], r4[:, :, :], gam_sb[:, :])
                    osb = osbp.tile([P, G, C], f32, tag="osb", name=f"osb{s}")
                    nc.vector.tensor_mul(
                        osb[:, :, :],
                        po4[:, :, 0:C],
                        r2[:, :, :].to_broadcast([P, G, C]),
                    )
                    nc.gpsimd.tensor_add(
                        osb[:, :, :], osb[:, :, :], xqT_sb[:, ts(s, G), :]
                    )
                    nc.sync.dma_start(
                        out=out_d.ap().rearrange("(t p) c -> p t c", p=P)[
                            :, ts(s, G), :
                        ],
                        in_=osb[:, :, :],
                    )

    nc.compile()
    return nc


def _host_prep(inputs):
    """Common host-side input preparation for all variants."""
    x = np.ascontiguousarray(np.asarray(inputs["x"], dtype=np.float32))
    Wq = np.asarray(inputs["Wq"], dtype=np.float32)
    Wk = np.asarray(inputs["Wk"], dtype=np.float32)
    Wv = np.asarray(inputs["Wv"], dtype=np.float32)
    bq = np.ascontiguousarray(np.asarray(inputs["bq"], dtype=np.float32))
    bk = np.ascontiguousarray(np.asarray(inputs["bk"], dtype=np.float32))
    bv = np.ascontiguousarray(np.asarray(inputs["bv"], dtype=np.float32))
    gamma = np.ascontiguousarray(np.asarray(inputs["gamma"], dtype=np.float32))

    xf = x.reshape(B, C, N)
    wqT = np.ascontiguousarray(np.tile(Wq.T, (1, G)))
    wkT = np.ascontiguousarray(np.tile(Wk.T, (1, G)))
    bqp = np.ascontiguousarray(np.tile(bq, G))
    bkp = np.ascontiguousarray(np.tile(bk, G))
    wvT = np.ascontiguousarray(
        np.concatenate([Wv.T, np.zeros((C, CP - C), np.float32)], axis=1)
    )
    bvp = np.concatenate(
        [bv, np.ones((1,), np.float32), np.zeros((CP - C - 1,), np.float32)]
    )

    in_maps = []
    for core in range(NCORES):
        b, half = divmod(core, HALVES)
        xroll = np.ascontiguousarray(np.roll(xf[b], -half * NI, axis=1))
        in_maps.append(
            {
                "xb": xroll,
                "xqT": np.ascontiguousarray(xroll[:, 0:NI].T),
                "wqT": wqT,
                "wkT": wkT,
                "wvT": wvT,
                "bq": bqp,
                "bk": bkp,
                "bv": bvp,
                "gamma": gamma,
            }
        )
    return in_maps


def kernel(**inputs):
    global LAST_RESULT
    from concourse.bass_utils import run_bass_kernel_spmd

    if "nc" not in _CACHE:
        _CACHE["nc"] = _build_program()
    nc = _CACHE["nc"]

    in_maps = _host_prep(inputs)

    trace = bool(os.environ.get("KERNEL_TRACE"))
    kwargs = {}
    if trace and os.environ.get("KERNEL_TRACE_ALL"):
        kwargs["trace_cores"] = list(range(NCORES))
        kwargs["stitch_traces"] = True
    res = run_bass_kernel_spmd(
        nc, in_maps, core_ids=list(range(NCORES)), trace=trace, **kwargs
    )
    LAST_RESULT = res

    out = np.empty((B, C, N), dtype=np.float32)
    for core in range(NCORES):
        b, half = divmod(core, HALVES)
        out[b][:, half * NI : (half + 1) * NI] = res.results[core]["out"].T
    return out.reshape(B, C, H, W)


# revision 12
# speedup vs baseline: 13.5774x; 10.4866x over previous
"""AttentionBlock kernel for Trainium2 (Bass/Tile), 8 NeuronCores.

Reference computation (B=4, C=256, H=W=64, Cqk=32, N=H*W=4096):
    q = Wq @ x + bq; k = Wk @ x + bk; v = Wv @ x + bv      (1x1 convs)
    energy[b,i,j] = sum_c q[b,c,i] k[b,c,j]
    attn = softmax(energy, axis=-1)
    out[b,c,i] = sum_j v[b,c,j] attn[b,i,j]
    result = gamma * out + x

Sharding: 8 cores = (batch b in 0..3) x (query-row half in 0..1).
Each core computes 2048 of the 4096 attention rows for one batch image;
the small conv weights are replicated.

Per-core design:
  - energy is computed TRANSPOSED: energyT[j, i] with j on partitions.
    The softmax denominator s_i comes from a ones-column appended to vT
    in the attn@v matmul (outT[:, 256] = s_i), so no partition-axis
    reduction is ever needed. No max-subtraction (|energy| <= ~45; exp
    stays in fp32 range; softmax ratios exact).
  - PE dtypes: all f32 data is bitcast to float32r at matmul use sites
    (1 PE cycle/column at N>=256 vs 4 for fp32; ~2^-11 relative error).
    exp() output is bf16 so the attn@v stationary operand gets the
    compiler's fast-weight-load (LDWEIGHTS fully hidden under the
    260-column f32r vT stream); the vT stream itself stays f32r for
    precision on the value side.
  - PACK_E: the energy matmul has contraction Cqk=32, so 4 j-chunks run
    concurrently in the PE array as 32-row tiles (tile_position row
    packing). q/k are built 4x-replicated along partitions by tiling the
    projection weights host-side.
  - The final output is produced TRANSPOSED ([NI, C]) and transposed on
    the host during unshard: the per-strip evacuation is then two large
    batched ops (scale by gamma/s_i with a stride-0 broadcast AP, then
    residual-add of host-supplied xqT) instead of 64 small per-block
    ops plus 128 PE transposes.
  - Strip-level PSUM accumulators [128, 4, 512] keep each attn@v matmul
    output bank-aligned (u * 2KB) while letting the evacuation read all
    four blocks in one instruction.
"""

import os

import numpy as np

B, C, H, W = 4, 256, 64, 64
CQK = 32
N = H * W                      # 4096
NCORES = 8
HALVES = 2                     # query-row halves per batch
NI = N // HALVES               # 2048 rows per core
P = 128                        # SBUF partitions
CC = C // P                    # 2 channel chunks
NJ = N // P                    # 32 key/value chunks
SW = 512                       # i-strip width
NSTRIP = NI // SW              # strips per core
PW = 512                       # projection tile width
NT_K = N // PW                 # k-proj tiles
NT_Q = NI // PW                # q-proj tiles
CP = C + 4                     # vT width: 256 v-ch + ones col + pad (f32r %4)
G = 4                          # row-packing group size (128 / CQK)
TI = NI // P                   # 16 i-tiles of 128 rows
QKP = P                        # q/k tiles replicated 4x along partitions

# float32r on all PE inputs (bitcast at use); attn@v operands (exp output
# and vT) in bf16 for fast-weight-load — walrus requires both matmul
# operands to share a transfer type when either is f32/f32r, so the pair
# switches together ("bfloat16" or "float32r").
_AV_DT = os.environ.get("KERNEL_AV_DT", "bfloat16")
_F32R = bool(int(os.environ.get("KERNEL_F32R", "1")))
# Compile walrus with --enable-ldw-opt=true: pipelines LDWEIGHTS under the
# previous matmul's stream.
_LDW_OPT = bool(int(os.environ.get("KERNEL_LDW_OPT", "1")))
# Software-pipeline the attn@v stage one group behind the energy/exp stage.
_SWP = bool(int(os.environ.get("KERNEL_SWP", "1")))
# Benchmark-only: repeat the computation R times in a hardware loop.
_REPEAT = int(os.environ.get("KERNEL_REPEAT", "1"))
# Benchmark-only ablations: run fewer attention strips / projection tiles
# (output is then wrong — used to attribute steady-state time per stage).
_NSTRIP = int(os.environ.get("KERNEL_NSTRIP", str(NSTRIP)))
_NPROJ = os.environ.get("KERNEL_NPROJ", "1") != "0"

_CACHE = {}
LAST_RESULT = None


def _enable_ldw_opt():
    """Recompile walrus flag --enable-ldw-opt=false -> true (in-process)."""
    import functools

    import concourse.bass_utils as bu

    if getattr(bu, "_ldw_opt_patched", False):
        return
    orig = bu.bir_verify_and_optimise

    @functools.wraps(orig)
    def patched(tmpdir, inp="bir.json", outp="file.neff", arch=None, *, dve_root=None):
        real = bu.run_command

        def hook(cmd, **kw):
            cmd = [
                "--enable-ldw-opt=true" if c == "--enable-ldw-opt=false" else c
                for c in cmd
            ]
            return real(cmd, **kw)

        bu.run_command = hook
        try:
            return orig(tmpdir, inp, outp, arch, dve_root=dve_root)
        finally:
            bu.run_command = real

    bu.bir_verify_and_optimise = patched
    bu._ldw_opt_patched = True


def _build_program():
    import contextlib

    if _LDW_OPT:
        _enable_ldw_opt()

    import concourse.bacc as bacc
    import concourse.bass as bass
    import concourse.mybir as mybir
    import concourse.tile as tile
    from concourse.bass import ts

    f32 = mybir.dt.float32
    f32r = mybir.dt.float32r if _F32R else mybir.dt.float32
    av_bf16 = _AV_DT == "bfloat16"
    av_dt = mybir.dt.bfloat16 if av_bf16 else f32r
    AF = mybir.ActivationFunctionType

    nc = bacc.Bacc("TRN2", target_bir_lowering=False, debug=False)

    xb_d = nc.dram_tensor("xb", [C, N], f32r, kind="ExternalInput")
    xqT_d = nc.dram_tensor("xqT", [NI, C], f32, kind="ExternalInput")
    wqT_d = nc.dram_tensor("wqT", [C, QKP], f32r, kind="ExternalInput")
    wkT_d = nc.dram_tensor("wkT", [C, QKP], f32r, kind="ExternalInput")
    wvT_d = nc.dram_tensor("wvT", [C, CP], f32r, kind="ExternalInput")
    bq_d = nc.dram_tensor("bq", [QKP], f32, kind="ExternalInput")
    bk_d = nc.dram_tensor("bk", [QKP], f32, kind="ExternalInput")
    bv_d = nc.dram_tensor("bv", [CP], f32, kind="ExternalInput")
    gam_d = nc.dram_tensor("gamma", [1], f32, kind="ExternalInput")
    out_d = nc.dram_tensor("out", [NI, C], f32, kind="ExternalOutput")

    with tile.TileContext(nc) as tc:
        with (
            tc.tile_pool(name="consts", bufs=1) as consts,
            tc.tile_pool(name="sb", bufs=1) as sb,
            tc.tile_pool(name="evac", bufs=2) as evac,
            tc.tile_pool(name="osbp", bufs=2) as osbp,
            tc.tile_pool(name="expp", bufs=2) as expp,
            tc.tile_pool(name="psE", bufs=1, space="PSUM") as psE,
            tc.tile_pool(name="psO", bufs=1, space="PSUM") as psO,
        ):
            # ---- constants / weights ----
            wq_sb = consts.tile([P, CC, QKP], f32r)
            nc.sync.dma_start(
                out=wq_sb[:, :, :],
                in_=wqT_d.ap().rearrange("(cc p) o -> p cc o", p=P),
            )
            wk_sb = consts.tile([P, CC, QKP], f32r)
            nc.sync.dma_start(
                out=wk_sb[:, :, :],
                in_=wkT_d.ap().rearrange("(cc p) o -> p cc o", p=P),
            )
            wv_sb = consts.tile([P, CC, CP], f32r)
            nc.sync.dma_start(
                out=wv_sb[:, :, :],
                in_=wvT_d.ap().rearrange("(cc p) c -> p cc c", p=P),
            )

            bq_sb = consts.tile([QKP, 1], f32)
            nc.gpsimd.dma_start(
                out=bq_sb[:, :], in_=bass.AP(bq_d, 0, [[1, QKP], [1, 1]])
            )
            bk_sb = consts.tile([QKP, 1], f32)
            nc.gpsimd.dma_start(
                out=bk_sb[:, :], in_=bass.AP(bk_d, 0, [[1, QKP], [1, 1]])
            )
            # bv broadcast along partitions (trailing 1.0 = ones column)
            bvb_sb = consts.tile([P, CP], f32)
            nc.gpsimd.dma_start(
                out=bvb_sb[:, :], in_=bass.AP(bv_d, 0, [[0, P], [1, CP]])
            )
            gam_sb = consts.tile([P, 1], f32)
            nc.gpsimd.dma_start(
                out=gam_sb[:, :], in_=bass.AP(gam_d, 0, [[0, P], [1, 1]])
            )

            rep = (
                tc.For_i(0, _REPEAT, 1) if _REPEAT > 1 else contextlib.nullcontext()
            )
            with rep:
                # ---- activations ----
                # x arrives column-rotated so this core's 2048 query columns
                # are always cols 0:NI (attention is permutation-invariant
                # over key/value positions). 4 DMAs to spread across queues.
                xb_sb = sb.tile([P, CC, N], f32r)
                xb_src = xb_d.ap().rearrange("(cc p) n -> p cc n", p=P)
                for d in range(4):
                    nc.sync.dma_start(
                        out=xb_sb[:, :, ts(d, N // 4)],
                        in_=xb_src[:, :, ts(d, N // 4)],
                    )
                # xqT: the query block of x, pre-transposed host-side, for
                # the residual add in the transposed output space.
                xqT_sb = sb.tile([P, TI, C], f32)
                xqT_src = xqT_d.ap().rearrange("(t p) c -> p t c", p=P)
                for d in range(2):
                    nc.gpsimd.dma_start(
                        out=xqT_sb[:, ts(d, TI // 2), :],
                        in_=xqT_src[:, ts(d, TI // 2), :],
                    )

                q_sb = sb.tile([QKP, NI], f32r)
                k_sb = sb.tile([QKP, N], f32r)
                vt_sb = sb.tile([P, NJ, CP], av_dt)

                # ---- projections (PSUM tiles ping-pong pe/po banks) ----
                # k = Wk @ xb + bk (4x-replicated on partitions)
                ptag = [0]

                def proj_ps(shape, name):
                    t = "pe" if ptag[0] % 2 == 0 else "po"
                    ptag[0] += 1
                    return (psE if t == "pe" else psO).tile(
                        [P, G, SW], f32, tag=t, name=name
                    )[:, 0, : shape[1]]

                for t in range(NT_K if _NPROJ else 0):
                    ps = proj_ps([QKP, PW], f"psk{t}")
                    for cc in range(CC):
                        nc.tensor.matmul(
                            ps,
                            wk_sb[:, cc, :],
                            xb_sb[:, cc, ts(t, PW)],
                            start=(cc == 0),
                            stop=(cc == CC - 1),
                        )
                    nc.vector.tensor_scalar_add(
                        k_sb[:, ts(t, PW)], ps, bk_sb[:, :]
                    )
                # q = Wq @ xq + bq
                for t in range(NT_Q if _NPROJ else 0):
                    ps = proj_ps([QKP, PW], f"psq{t}")
                    for cc in range(CC):
                        nc.tensor.matmul(
                            ps,
                            wq_sb[:, cc, :],
                            xb_sb[:, cc, ts(t, PW)],
                            start=(cc == 0),
                            stop=(cc == CC - 1),
                        )
                    nc.vector.tensor_scalar_add(
                        q_sb[:, ts(t, PW)], ps, bq_sb[:, :]
                    )
                # vT = (Wv @ xb + bv).T -> [4096, CP]; wvT's zero columns
                # plus bv's trailing 1.0 produce the ones column that yields
                # the softmax denominator in the attn@v matmul.
                for j in range(NJ if _NPROJ else 0):
                    ps = proj_ps([P, CP], f"psv{j}")
                    for cc in range(CC):
                        nc.tensor.matmul(
                            ps,
                            xb_sb[:, cc, ts(j, P)],
                            wv_sb[:, cc, :],
                            start=(cc == 0),
                            stop=(cc == CC - 1),
                        )
                    nc.vector.tensor_add(vt_sb[:, j, :], ps, bvb_sb[:, :])

                # ---- attention strips ----
                for s in range(_NSTRIP):
                    po4 = psO.tile([P, G, SW], f32, tag="po", name=f"po{s}")

                    def emit_o(ex4_p, t_p):
                        for g in range(G):
                            j = G * t_p + g
                            for u in range(SW // P):
                                nc.tensor.matmul(
                                    po4[:, u, 0:CP],
                                    ex4_p[:, g, ts(u, P)],
                                    vt_sb[:, j, :],
                                    start=(t_p == 0 and g == 0),
                                    stop=(t_p == NJ // G - 1 and g == G - 1),
                                )

                    pending = None
                    for t in range(NJ // G):
                        pe4 = psE.tile([P, G, SW], f32, tag="pe")
                        for g in range(G):
                            j = G * t + g
                            nc.tensor.matmul(
                                pe4[:, g, :],
                                k_sb[32 * g : 32 * (g + 1), ts(j, P)],
                                q_sb[32 * g : 32 * (g + 1), ts(s, SW)],
                                start=True,
                                stop=True,
                                tile_position=(32 * g, 0),
                            )
                        ex4 = expp.tile([P, G, SW], av_dt, tag="ex")
                        nc.scalar.activation(ex4[:, :, :], pe4[:, :, :], AF.Exp)
                        if not _SWP:
                            emit_o(ex4, t)
                        else:
                            if pending is not None:
                                emit_o(*pending)
                            pending = (ex4, t)
                    if _SWP and pending is not None:
                        emit_o(*pending)

                    # ---- strip evacuation (batched, transposed space) ----
                    # r2 = gamma / s_i ; osb = po4 * r2 + xqT
                    r4 = evac.tile([P, G, 1], f32, tag="r4")
                    nc.vector.reciprocal(r4[:, :, :], po4[:, :, C : C + 1])
                    r2 = evac.tile([P, G, 1], f32, tag="r2")
                    nc.vector.tensor_scalar_mul(r2[:, :, :], r4[:, :, :], gam_sb[:, :])
                    osb = osbp.tile([P, G, C], f32, tag="osb", name=f"osb{s}")
                    nc.vector.tensor_mul(
                        osb[:, :, :],
                        po4[:, :, 0:C],
                        r2[:, :, :].to_broadcast([P, G, C]),
                    )
                    nc.gpsimd.tensor_add(
                        osb[:, :, :], osb[:, :, :], xqT_sb[:, ts(s, G), :]
                    )
                    nc.sync.dma_start(
                        out=out_d.ap().rearrange("(t p) c -> p t c", p=P)[
                            :, ts(s, G), :
                        ],
                        in_=osb[:, :, :],
                    )

    nc.compile()
    return nc


def _host_prep(inputs):
    """Common host-side input preparation for all variants."""
    x = np.ascontiguousarray(np.asarray(inputs["x"], dtype=np.float32))
    Wq = np.asarray(inputs["Wq"], dtype=np.float32)
    Wk = np.asarray(inputs["Wk"], dtype=np.float32)
    Wv = np.asarray(inputs["Wv"], dtype=np.float32)
    bq = np.ascontiguousarray(np.asarray(inputs["bq"], dtype=np.float32))
    bk = np.ascontiguousarray(np.asarray(inputs["bk"], dtype=np.float32))
    bv = np.ascontiguousarray(np.asarray(inputs["bv"], dtype=np.float32))
    gamma = np.ascontiguousarray(np.asarray(inputs["gamma"], dtype=np.float32))

    xf = x.reshape(B, C, N)
    wqT = np.ascontiguousarray(np.tile(Wq.T, (1, G)))
    wkT = np.ascontiguousarray(np.tile(Wk.T, (1, G)))
    bqp = np.ascontiguousarray(np.tile(bq, G))
    bkp = np.ascontiguousarray(np.tile(bk, G))
    wvT = np.ascontiguousarray(
        np.concatenate([Wv.T, np.zeros((C, CP - C), np.float32)], axis=1)
    )
    bvp = np.concatenate(
        [bv, np.ones((1,), np.float32), np.zeros((CP - C - 1,), np.float32)]
    )

    in_maps = []
    for core in range(NCORES):
        b, half = divmod(core, HALVES)
        xroll = np.ascontiguousarray(np.roll(xf[b], -half * NI, axis=1))
        in_maps.append(
            {
                "xb": xroll,
                "xqT": np.ascontiguousarray(xroll[:, 0:NI].T),
                "wqT": wqT,
                "wkT": wkT,
                "wvT": wvT,
                "bq": bqp,
                "bk": bkp,
                "bv": bvp,
                "gamma": gamma,
            }
        )
    return in_maps


def kernel(**inputs):
    global LAST_RESULT
    from concourse.bass_utils import run_bass_kernel_spmd

    if "nc" not in _CACHE:
        _CACHE["nc"] = _build_program()
    nc = _CACHE["nc"]

    in_maps = _host_prep(inputs)

    trace = bool(os.environ.get("KERNEL_TRACE"))
    kwargs = {}
    if trace and os.environ.get("KERNEL_TRACE_ALL"):
        kwargs["trace_cores"] = list(range(NCORES))
        kwargs["stitch_traces"] = True
    res = run_bass_kernel_spmd(
        nc, in_maps, core_ids=list(range(NCORES)), trace=trace, **kwargs
    )
    LAST_RESULT = res

    out = np.empty((B, C, N), dtype=np.float32)
    for core in range(NCORES):
        b, half = divmod(core, HALVES)
        out[b][:, half * NI : (half + 1) * NI] = res.results[core]["out"].T
    return out.reshape(B, C, H, W)
